# revision 23
# baseline (speedup 1.0000x reference)
"""Multi-head attention kernel for Trainium2, 8-way head-sharded.

Reference computation (see problem spec):
    kval = einsum('bcm,hmk->bhck', kvinput, wk)
    qval = einsum('bcm,hmk->bhck', qinput,  wq)
    vval = einsum('bcm,hmv->bhcv', kvinput, wv)
    alogit[b,h,c,d] = sum_k kval[c,k] qval[d,k]          (c=key, d=query)
    alogit += mask          mask[c,d] = 0 if c>d else -100
    att = softmax(alogit / sqrt(K), axis=c)
    pre = einsum('bhcd,bhcv->bhdv', att, vval)
    out = einsum('bhcv,hvm->bcm', pre, wo)

Sharding: H=16 heads -> 2 heads per core on 8 cores.  Every core gets the
full kvinput/qinput plus its own 2-head slice of wq/wk/wv/wo, computes its
partial out (summed over its heads), and per-batch on-device ReduceScatters
produce disjoint output shards (host concatenates).

Per-core pipeline (bf16 matmuls, fp32 softmax internals):
  Phase A (per batch b): cast input tiles to bf16 (GpSimd), XBAR
    DMA-transpose [c,m]->[m,c], then project: qvalT/kvalT [k2h=128, C]
    (two heads stacked on partitions) and vvalT [v2h=128, C], which is
    DMA-transposed to vval [c, v] per head with a ones column appended
    (for softmax denominators).
  Phase B (per b, per 512-wide query tile dt): for each 128-wide key chunk:
    S = kvalT.T @ qvalT -> [c=128, d=512] per head (PSUM fp32),
    mask add where the tile crosses the diagonal (DVE),
    P = exp(S/8 + bias) (ACT, both heads in one op, bf16 out),
    pre[v|1, d] += [vval|1].T @ P (PE, accumulated over key chunks; row V
    of the result is the softmax denominator).
    Then broadcast the denominator across partitions with a tiny f32r
    matmul, reciprocal_approx_fast, multiply-normalize into preT (bf16),
    and project: out[d,m] += preT.T @ wo with both heads stacked on the
    contraction axis.
"""

import numpy as np

import concourse.bacc as bacc
import concourse.bass as bass
import concourse.mybir as mybir
import concourse.tile as tile
from concourse.bass_utils import run_bass_kernel_spmd

# Problem dims (hardcoded per harness contract).
B, C, M, H, K, V = 2, 2048, 1024, 16, 64, 64
NCORES = 8
HPC = H // NCORES  # heads per core
P = 128

F32 = mybir.dt.float32
F32R = mybir.dt.float32r
BF16 = mybir.dt.bfloat16
EXP = mybir.ActivationFunctionType.Exp
ADD = mybir.AluOpType.add
MULT = mybir.AluOpType.mult

CT_W = 256              # phase-A c-tile width
N_CT = C // CT_W        # 8
N_CS = CT_W // P        # 2 c-subtiles per phase-A tile
N_MC = M // P           # 8 contraction chunks
DT_W = 512              # query tile width
N_DT = C // DT_W        # 4
N_CC = C // P           # 16 key chunks
SCALE = float(1.0 / np.sqrt(np.float32(K)))   # 0.125
MASK_BIAS = -100.0 * SCALE                    # -12.5

# Skip (QK, exp, AV) for fully-masked key chunks (c <= d everywhere) except in
# the last query tile, where few/no unmasked keys exist and the masked terms
# carry the softmax.  Dropped terms change the result by < ~2e-5 relative.
SKIP_MASKED = True


def _tile_class(cc: int, dt: int):
    """Classify key-chunk cc (128-wide) vs query-tile dt (512-wide)."""
    t = cc * P - dt * DT_W
    if t >= DT_W:
        return "keep", None
    if t >= 0:
        return "cross", t // P
    return "masked", None


def make_masks() -> np.ndarray:
    """Crossing masks for offsets t = c0-d0 in {0,128,256,384}.

    mask[t//128, i, j] = 0 if (t+i) > j else -100   (c=c0+i, d=d0+j)
    """
    i = np.arange(P)[:, None]
    j = np.arange(DT_W)[None, :]
    return np.stack(
        [np.where(t + i > j, 0.0, -100.0).astype(np.float32)
         for t in (0, 128, 256, 384)]
    )


def build_module() -> bass.Bass:
    nc = bacc.Bacc("TRN2", num_devices=NCORES)

    kv_d = nc.dram_tensor("kvinput", [B, C, M], F32, kind="ExternalInput")
    q_d = nc.dram_tensor("qinput", [B, C, M], F32, kind="ExternalInput")
    wq_d = nc.dram_tensor("wq", [HPC, M, K], F32, kind="ExternalInput")
    wk_d = nc.dram_tensor("wk", [HPC, M, K], F32, kind="ExternalInput")
    wv_d = nc.dram_tensor("wv", [HPC, M, V], F32, kind="ExternalInput")
    wo_d = nc.dram_tensor("wo", [HPC, V, M], F32, kind="ExternalInput")
    mk_d = nc.dram_tensor("masks", [4, P, DT_W], F32, kind="ExternalInput")
    # Per-core output shard: ReduceScatter leaves rank r with slice r of the
    # flattened [C*M] batch output.
    out_d = nc.dram_tensor("out", [B, C * M // NCORES], F32, kind="ExternalOutput")

    with (
        tile.TileContext(nc) as tc,
        tc.tile_pool(name="const", bufs=1) as constp,
        tc.tile_pool(name="perb", bufs=2) as perb,
        tc.tile_pool(name="pha2", bufs=2) as pha2,
        tc.tile_pool(name="work", bufs=3) as work,
        tc.tile_pool(name="work2", bufs=2) as work2,
        tc.tile_pool(name="ps_s", bufs=2, space="PSUM") as ps_s,
        tc.tile_pool(name="ps_b", bufs=4, space="PSUM") as ps_b,
        tc.tile_pool(name="dram", bufs=1, space="DRAM") as dram,
    ):
        # ---- constants ----
        ones_row = constp.tile([P, V], F32R)
        nc.vector.tensor_copy(ones_row, nc.const_aps.tensor(1.0, (P, V)))
        mbias = constp.tile([P, 1], F32)
        nc.vector.memset(mbias, MASK_BIAS)

        wq_sb = constp.tile([P, N_MC, HPC * K], BF16)
        wk_sb = constp.tile([P, N_MC, HPC * K], BF16)
        wv_sb = constp.tile([P, N_MC, HPC * V], BF16)
        wo_sb = constp.tile([P, M], BF16)
        for wi, (w_d, w_sb) in enumerate(
            ((wq_d, wq_sb), (wk_d, wk_sb), (wv_d, wv_sb))
        ):
            for h in range(HPC):
                wstage = constp.tile([P, N_MC, K], F32, tag=f"wstage{wi}{h}")
                nc.sync.dma_start(
                    wstage, w_d[h].rearrange("(mo p) k -> p mo k", p=P)
                )
                w3 = w_sb.rearrange("p mo (h k) -> p mo h k", h=HPC)
                nc.vector.tensor_copy(w3[:, :, h, :], wstage)
        wostage = constp.tile([P, M], F32, tag="wostage")
        nc.sync.dma_start(wostage, wo_d[:].rearrange("h v m -> (h v) m"))
        nc.vector.tensor_copy(wo_sb, wostage)
        masks_sb = constp.tile([P, 4, DT_W], F32)
        for t in range(4):
            nc.sync.dma_start(masks_sb[:, t, :], mk_d[t])

        partial = dram.tile([B, C * M], F32)
        shard = dram.tile([B, C * M // NCORES], F32)

        for b in range(B):
            qvalT = perb.tile([P, C], BF16, tag="qvalT")   # [k2h, c]
            kvalT = perb.tile([P, C], BF16, tag="kvalT")   # [k2h, c]
            vvalT = perb.tile([P, C], BF16, tag="vvalT")   # [v2h, c]
            # [c, cc, v|1 | v|1]: head0 cols 0:64 + ones col 64,
            #                     head1 cols 65:129 + ones col 129
            vv = perb.tile([P, N_CC, 2 * V + 2], BF16, tag="vv")
            preT = perb.tile([P, C], BF16, tag="preT")     # [v2h, d] normalized

            nc.vector.tensor_copy(
                vv[:, :, V], nc.const_aps.tensor(1.0, (P, N_CC))
            )
            nc.vector.tensor_copy(
                vv[:, :, 2 * V + 1], nc.const_aps.tensor(1.0, (P, N_CC))
            )

            # ---------- Phase A: cast + transpose inputs, QKV projections ----
            for ct in range(N_CT):
                c0 = ct * CT_W
                q_nat = pha2.tile([P, N_CS, M], F32, tag="q_nat")
                kv_nat = pha2.tile([P, N_CS, M], F32, tag="kv_nat")
                nc.sync.dma_start(
                    q_nat, q_d[b, c0:c0 + CT_W, :].rearrange("(cs p) m -> p cs m", p=P)
                )
                nc.sync.dma_start(
                    kv_nat, kv_d[b, c0:c0 + CT_W, :].rearrange("(cs p) m -> p cs m", p=P)
                )
                q_bf = pha2.tile([P, N_CS, M], BF16, tag="q_bf")
                kv_bf = pha2.tile([P, N_CS, M], BF16, tag="kv_bf")
                # split the fp32->bf16 casts between DVE and ACT so neither
                # engine (nor GpSimd, which must stay free to issue the
                # collectives without blocking) becomes the phase-A bottleneck
                if ct % 2 == 0:
                    nc.vector.tensor_copy(q_bf, q_nat)
                    nc.scalar.copy(kv_bf, kv_nat)
                else:
                    nc.scalar.copy(q_bf, q_nat)
                    nc.vector.tensor_copy(kv_bf, kv_nat)

                qT = pha2.tile([P, N_MC, CT_W], BF16, tag="qT")    # [m, mc, c]
                kvT = pha2.tile([P, N_MC, CT_W], BF16, tag="kvT")
                for cs in range(N_CS):
                    nc.scalar.dma_start_transpose(
                        qT[:, :, cs * P:(cs + 1) * P], q_bf[:, cs, :]
                    )
                    nc.scalar.dma_start_transpose(
                        kvT[:, :, cs * P:(cs + 1) * P], kv_bf[:, cs, :]
                    )

                acc_q = ps_b.tile([P, DT_W], F32, tag="pb")
                acc_k = ps_b.tile([P, DT_W], F32, tag="pb")
                acc_v = ps_b.tile([P, DT_W], F32, tag="pb")
                for mc in range(N_MC):
                    nc.tensor.matmul(
                        acc_q[:, 0:CT_W], lhsT=wq_sb[:, mc], rhs=qT[:, mc, :],
                        start=(mc == 0), stop=(mc == N_MC - 1),
                    )
                    nc.tensor.matmul(
                        acc_k[:, 0:CT_W], lhsT=wk_sb[:, mc], rhs=kvT[:, mc, :],
                        start=(mc == 0), stop=(mc == N_MC - 1),
                    )
                    nc.tensor.matmul(
                        acc_v[:, 0:CT_W], lhsT=wv_sb[:, mc], rhs=kvT[:, mc, :],
                        start=(mc == 0), stop=(mc == N_MC - 1),
                    )
                nc.vector.tensor_copy(qvalT[:, c0:c0 + CT_W], acc_q[:, 0:CT_W])
                nc.vector.tensor_copy(kvalT[:, c0:c0 + CT_W], acc_k[:, 0:CT_W])
                nc.vector.tensor_copy(vvalT[:, c0:c0 + CT_W], acc_v[:, 0:CT_W])

            # vvalT [v2h, c] -> vval [c, v] per head via XBAR transpose.
            vvs = work2.tile([P, N_CC, P], BF16, tag="vvs")
            nc.scalar.dma_start_transpose(vvs, vvalT)
            nc.vector.tensor_copy(vv[:, :, 0:V], vvs[:, :, 0:V])
            nc.vector.tensor_copy(vv[:, :, V + 1:2 * V + 1], vvs[:, :, V:2 * V])

            # ---------- Phase B: attention + output projection ----------
            for dt in range(N_DT):
                d0 = dt * DT_W
                pre0 = ps_b.tile([P, DT_W], F32, tag="pb")  # [v|sum, d] head0
                pre1 = ps_b.tile([P, DT_W], F32, tag="pb")

                ccs = [cc for cc in range(N_CC)
                       if not (SKIP_MASKED and dt < N_DT - 1
                               and _tile_class(cc, dt)[0] == "masked")]

                def emit_s(cc):
                    s01 = ps_s.tile([P, 1024], F32, tag="s")
                    nc.tensor.matmul(
                        s01[:, 0:DT_W],
                        lhsT=kvalT[0:K, cc * P:(cc + 1) * P],
                        rhs=qvalT[0:K, d0:d0 + DT_W],
                        start=True, stop=True,
                    )
                    nc.tensor.matmul(
                        s01[:, DT_W:2 * DT_W],
                        lhsT=kvalT[K:2 * K, cc * P:(cc + 1) * P],
                        rhs=qvalT[K:2 * K, d0:d0 + DT_W],
                        start=True, stop=True,
                    )
                    return s01

                # software pipeline: S(cc+1) is emitted (and thus issued on
                # PE) before AV(cc), so PE computes the next tile's scores
                # while ACT runs exp on the current one.
                s_next = emit_s(ccs[0])
                for i, cc in enumerate(ccs):
                    cls, mi = _tile_class(cc, dt)
                    s01 = s_next
                    if i + 1 < len(ccs):
                        s_next = emit_s(ccs[i + 1])
                    if cls == "cross":
                        s2 = s01.rearrange("p (g d) -> p g d", g=2)
                        nc.vector.tensor_tensor(
                            s2, s2,
                            masks_sb[:, mi, None, :].to_broadcast((P, 2, DT_W)),
                            ADD,
                        )
                    bias = mbias if cls == "masked" else 0.0
                    p01 = work.tile([P, 1024], BF16, tag="p01")
                    nc.scalar.activation(p01, s01, EXP, bias=bias, scale=SCALE)

                    first, last = (i == 0), (i == len(ccs) - 1)
                    nc.tensor.matmul(
                        pre0[0:V + 1, :],
                        lhsT=vv[:, cc, 0:V + 1],
                        rhs=p01[:, 0:DT_W],
                        start=first, stop=last,
                    )
                    nc.tensor.matmul(
                        pre1[0:V + 1, :],
                        lhsT=vv[:, cc, V + 1:2 * V + 2],
                        rhs=p01[:, DT_W:2 * DT_W],
                        start=first, stop=last,
                    )

                # normalize: preT[v, d] = pre[v, d] / pre[V, d]
                srow = work2.tile([P, 2, DT_W], F32R, tag="srow")
                nc.vector.tensor_copy(srow[V:V + 1, 0, :], pre0[V:V + 1, :])
                nc.vector.tensor_copy(srow[V:V + 1, 1, :], pre1[V:V + 1, :])
                for h, (pre_ps, dst_lo) in enumerate(((pre0, True), (pre1, False))):
                    bc = ps_b.tile([P, DT_W], F32, tag="pb")
                    nc.tensor.matmul(
                        bc[0:V, :],
                        lhsT=ones_row[V:V + 1, :],
                        rhs=srow[V:V + 1, h, :],
                        start=True, stop=True,
                    )
                    rb = work2.tile([P, DT_W], F32, tag="rb")
                    with nc.allow_low_precision(reason="softmax denominator"):
                        nc.vector.reciprocal_approx_fast(rb[0:V, :], bc[0:V, :])
                    if dst_lo:
                        nc.vector.tensor_tensor(
                            preT[0:V, d0:d0 + DT_W], pre_ps[0:V, :], rb[0:V, :], MULT
                        )
                    else:
                        pre1n = work2.tile([P, DT_W], BF16, tag="pre1n")
                        nc.vector.tensor_tensor(
                            pre1n[0:V, :], pre_ps[0:V, :], rb[0:V, :], MULT
                        )
                        # head1 rows live at partitions 0..63; shift to 64..127
                        nc.sync.dma_start(preT[V:2 * V, d0:d0 + DT_W], pre1n[0:V, :])

            # output projection: out[d, m] = sum_{v2h} preT[v2h, d] * wo[v2h, m]
            # Split into d-halves; each half's ReduceScatter is kicked as soon
            # as its output rows are in DRAM, overlapping the collective with
            # the remaining compute.
            pb2 = partial[b].rearrange("(half dd p mm) -> half p dd mm", half=2, p=P, mm=M)
            half_sub = C // P // 2
            for half in range(2):
                for dsl in range(half_sub):
                    ds_ = half * half_sub + dsl
                    for mt in range(M // DT_W):
                        o_ps = ps_b.tile([P, DT_W], F32, tag="pb")
                        nc.tensor.matmul(
                            o_ps,
                            lhsT=preT[:, ds_ * P:(ds_ + 1) * P],
                            rhs=wo_sb[:, mt * DT_W:(mt + 1) * DT_W],
                            start=True, stop=True,
                        )
                        o_sb = work.tile([P, DT_W], F32, tag="o_sb")
                        nc.vector.tensor_copy(o_sb, o_ps)
                        nc.sync.dma_start(
                            pb2[half, :, dsl, mt * DT_W:(mt + 1) * DT_W], o_sb
                        )
                hsz = C * M // 2
                ssz = hsz // NCORES
                nc.gpsimd.collective_compute(
                    "ReduceScatter",
                    ADD,
                    replica_groups=[list(range(NCORES))],
                    ins=[partial[b, half * hsz:(half + 1) * hsz].opt()],
                    outs=[shard[b, half * ssz:(half + 1) * ssz].opt()],
                )
                nc.sync.dma_start(
                    out_d[b, half * ssz:(half + 1) * ssz],
                    shard[b, half * ssz:(half + 1) * ssz],
                )

    nc.finalize()
    return nc


_CACHE: dict = {}


def run(inputs: dict, trace: bool = False, **spmd_kwargs):
    """Run on 8 cores; returns (BassKernelResults, assembled output)."""
    if "nc" not in _CACHE:
        _CACHE["nc"] = build_module()
    nc = _CACHE["nc"]

    kvinput = np.ascontiguousarray(np.asarray(inputs["kvinput"], dtype=np.float32))
    qinput = np.ascontiguousarray(np.asarray(inputs["qinput"], dtype=np.float32))
    wq = np.asarray(inputs["wq"], dtype=np.float32)
    wk = np.asarray(inputs["wk"], dtype=np.float32)
    wv = np.asarray(inputs["wv"], dtype=np.float32)
    wo = np.asarray(inputs["wo"], dtype=np.float32)
    masks = make_masks()

    in_maps = []
    for i in range(NCORES):
        h0 = i * HPC
        in_maps.append({
            "kvinput": kvinput,
            "qinput": qinput,
            "wq": np.ascontiguousarray(wq[h0:h0 + HPC]),
            "wk": np.ascontiguousarray(wk[h0:h0 + HPC]),
            "wv": np.ascontiguousarray(wv[h0:h0 + HPC]),
            "wo": np.ascontiguousarray(wo[h0:h0 + HPC]),
            "masks": masks,
        })

    res = run_bass_kernel_spmd(
        nc, in_maps, core_ids=list(range(NCORES)), trace=trace, **spmd_kwargs
    )
    # Each rank's "out" is [B, CM/8] = [B, 2 halves, CM/16]: slice r of each
    # half-batch ReduceScatter.
    qsz = C * M // (2 * NCORES)
    out = np.empty((B, 2, NCORES, qsz), dtype=np.float32)
    for r in range(NCORES):
        out[:, :, r, :] = res.results[r]["out"].reshape(B, 2, qsz)
    return res, out.reshape(B, C, M)


def kernel(**inputs: np.ndarray) -> np.ndarray:
    return run(inputs)[1]


# revision 24
# speedup vs baseline: 1.0711x; 1.0711x over previous
"""Multi-head attention kernel for Trainium2, 8-way head-sharded.

Reference computation (see problem spec):
    kval = einsum('bcm,hmk->bhck', kvinput, wk)
    qval = einsum('bcm,hmk->bhck', qinput,  wq)
    vval = einsum('bcm,hmv->bhcv', kvinput, wv)
    alogit[b,h,c,d] = sum_k kval[c,k] qval[d,k]          (c=key, d=query)
    alogit += mask          mask[c,d] = 0 if c>d else -100
    att = softmax(alogit / sqrt(K), axis=c)
    pre = einsum('bhcd,bhcv->bhdv', att, vval)
    out = einsum('bhcv,hvm->bcm', pre, wo)

Sharding: H=16 heads -> 2 heads per core on 8 cores.  Every core gets the
full kvinput/qinput plus its own 2-head slice of wq/wk/wv/wo, computes its
partial out (summed over its heads), and per-batch on-device ReduceScatters
produce disjoint output shards (host concatenates).

Per-core pipeline (bf16 matmuls, fp32 softmax internals):
  Phase A (per batch b): cast input tiles to bf16 (GpSimd), XBAR
    DMA-transpose [c,m]->[m,c], then project: qvalT/kvalT [k2h=128, C]
    (two heads stacked on partitions) and vvalT [v2h=128, C], which is
    DMA-transposed to vval [c, v] per head with a ones column appended
    (for softmax denominators).
  Phase B (per b, per 512-wide query tile dt): for each 128-wide key chunk:
    S = kvalT.T @ qvalT -> [c=128, d=512] per head (PSUM fp32),
    mask add where the tile crosses the diagonal (DVE),
    P = exp(S/8 + bias) (ACT, both heads in one op, bf16 out),
    pre[v|1, d] += [vval|1].T @ P (PE, accumulated over key chunks; row V
    of the result is the softmax denominator).
    Then broadcast the denominator across partitions with a tiny f32r
    matmul, reciprocal_approx_fast, multiply-normalize into preT (bf16),
    and project: out[d,m] += preT.T @ wo with both heads stacked on the
    contraction axis.
"""

import numpy as np

import concourse.bacc as bacc
import concourse.bass as bass
import concourse.mybir as mybir
import concourse.tile as tile
from concourse.bass_utils import run_bass_kernel_spmd

# Problem dims (hardcoded per harness contract).
B, C, M, H, K, V = 2, 2048, 1024, 16, 64, 64
NCORES = 8
HPC = H // NCORES  # heads per core
P = 128

F32 = mybir.dt.float32
F32R = mybir.dt.float32r
BF16 = mybir.dt.bfloat16
EXP = mybir.ActivationFunctionType.Exp
ADD = mybir.AluOpType.add
MULT = mybir.AluOpType.mult

CT_W = 256              # phase-A c-tile width
N_CT = C // CT_W        # 8
N_CS = CT_W // P        # 2 c-subtiles per phase-A tile
N_MC = M // P           # 8 contraction chunks
DT_W = 512              # query tile width
N_DT = C // DT_W        # 4
N_CC = C // P           # 16 key chunks
SCALE = float(1.0 / np.sqrt(np.float32(K)))   # 0.125
MASK_BIAS = -100.0 * SCALE                    # -12.5

# Skip (QK, exp, AV) for fully-masked key chunks (c <= d everywhere) except in
# the last query tile, where few/no unmasked keys exist and the masked terms
# carry the softmax.  Dropped terms change the result by < ~2e-5 relative.
SKIP_MASKED = True


def _tile_class(cc: int, dt: int):
    """Classify key-chunk cc (128-wide) vs query-tile dt (512-wide)."""
    t = cc * P - dt * DT_W
    if t >= DT_W:
        return "keep", None
    if t >= 0:
        return "cross", t // P
    return "masked", None


def make_masks() -> np.ndarray:
    """Crossing masks for offsets t = c0-d0 in {0,128,256,384}.

    mask[t//128, i, j] = 0 if (t+i) > j else -100   (c=c0+i, d=d0+j)
    """
    i = np.arange(P)[:, None]
    j = np.arange(DT_W)[None, :]
    return np.stack(
        [np.where(t + i > j, 0.0, -100.0).astype(np.float32)
         for t in (0, 128, 256, 384)]
    )


def build_module() -> bass.Bass:
    nc = bacc.Bacc("TRN2", num_devices=NCORES)

    kv_d = nc.dram_tensor("kvinput", [B, C, M], F32, kind="ExternalInput")
    q_d = nc.dram_tensor("qinput", [B, C, M], F32, kind="ExternalInput")
    wq_d = nc.dram_tensor("wq", [HPC, M, K], F32, kind="ExternalInput")
    wk_d = nc.dram_tensor("wk", [HPC, M, K], F32, kind="ExternalInput")
    wv_d = nc.dram_tensor("wv", [HPC, M, V], F32, kind="ExternalInput")
    wo_d = nc.dram_tensor("wo", [HPC, V, M], F32, kind="ExternalInput")
    mk_d = nc.dram_tensor("masks", [4, P, DT_W], F32, kind="ExternalInput")
    # Per-core output shard: ReduceScatter leaves rank r with slice r of the
    # flattened [C*M] batch output.
    out_d = nc.dram_tensor("out", [B, C * M // NCORES], F32, kind="ExternalOutput")

    with (
        tile.TileContext(nc) as tc,
        tc.tile_pool(name="const", bufs=1) as constp,
        tc.tile_pool(name="perb", bufs=2) as perb,
        tc.tile_pool(name="pha2", bufs=2) as pha2,
        tc.tile_pool(name="work", bufs=3) as work,
        tc.tile_pool(name="work2", bufs=2) as work2,
        tc.tile_pool(name="ps_s", bufs=2, space="PSUM") as ps_s,
        tc.tile_pool(name="ps_b", bufs=4, space="PSUM") as ps_b,
        tc.tile_pool(name="dram", bufs=1, space="DRAM") as dram,
    ):
        # ---- constants ----
        ones_row = constp.tile([P, V], F32R)
        nc.vector.tensor_copy(ones_row, nc.const_aps.tensor(1.0, (P, V)))
        mbias = constp.tile([P, 1], F32)
        nc.vector.memset(mbias, MASK_BIAS)

        wq_sb = constp.tile([P, N_MC, HPC * K], BF16)
        wk_sb = constp.tile([P, N_MC, HPC * K], BF16)
        wv_sb = constp.tile([P, N_MC, HPC * V], BF16)
        wo_sb = constp.tile([P, M], BF16)
        for wi, (w_d, w_sb) in enumerate(
            ((wq_d, wq_sb), (wk_d, wk_sb), (wv_d, wv_sb))
        ):
            for h in range(HPC):
                wstage = constp.tile([P, N_MC, K], F32, tag=f"wstage{wi}{h}")
                nc.sync.dma_start(
                    wstage, w_d[h].rearrange("(mo p) k -> p mo k", p=P)
                )
                w3 = w_sb.rearrange("p mo (h k) -> p mo h k", h=HPC)
                nc.vector.tensor_copy(w3[:, :, h, :], wstage)
        wostage = constp.tile([P, M], F32, tag="wostage")
        nc.sync.dma_start(wostage, wo_d[:].rearrange("h v m -> (h v) m"))
        nc.vector.tensor_copy(wo_sb, wostage)
        masks_sb = constp.tile([P, 4, DT_W], F32)
        for t in range(4):
            nc.sync.dma_start(masks_sb[:, t, :], mk_d[t])

        partial = dram.tile([B, C * M], F32)
        shard = dram.tile([B, C * M // NCORES], F32)

        for b in range(B):
            qvalT = perb.tile([P, C], BF16, tag="qvalT")   # [k2h, c]
            kvalT = perb.tile([P, C], BF16, tag="kvalT")   # [k2h, c]
            vvalT = perb.tile([P, C], BF16, tag="vvalT")   # [v2h, c]
            # [c, cc, v|1 | v|1]: head0 cols 0:64 + ones col 64,
            #                     head1 cols 65:129 + ones col 129
            vv = perb.tile([P, N_CC, 2 * V + 2], BF16, tag="vv")
            preT = perb.tile([P, C], BF16, tag="preT")     # [v2h, d] normalized

            nc.vector.tensor_copy(
                vv[:, :, V], nc.const_aps.tensor(1.0, (P, N_CC))
            )
            nc.vector.tensor_copy(
                vv[:, :, 2 * V + 1], nc.const_aps.tensor(1.0, (P, N_CC))
            )

            # ---------- Phase A: cast + transpose inputs, QKV projections ----
            for ct in range(N_CT):
                c0 = ct * CT_W
                q_nat = pha2.tile([P, N_CS, M], F32, tag="q_nat")
                kv_nat = pha2.tile([P, N_CS, M], F32, tag="kv_nat")
                nc.sync.dma_start(
                    q_nat, q_d[b, c0:c0 + CT_W, :].rearrange("(cs p) m -> p cs m", p=P)
                )
                nc.sync.dma_start(
                    kv_nat, kv_d[b, c0:c0 + CT_W, :].rearrange("(cs p) m -> p cs m", p=P)
                )
                q_bf = pha2.tile([P, N_CS, M], BF16, tag="q_bf")
                kv_bf = pha2.tile([P, N_CS, M], BF16, tag="kv_bf")
                # casts on DVE; ACT is reserved for exp + half the output
                # copies, GpSimd for issuing collectives without blocking
                nc.vector.tensor_copy(q_bf, q_nat)
                nc.vector.tensor_copy(kv_bf, kv_nat)

                qT = pha2.tile([P, N_MC, CT_W], BF16, tag="qT")    # [m, mc, c]
                kvT = pha2.tile([P, N_MC, CT_W], BF16, tag="kvT")
                for cs in range(N_CS):
                    nc.scalar.dma_start_transpose(
                        qT[:, :, cs * P:(cs + 1) * P], q_bf[:, cs, :]
                    )
                    nc.sync.dma_start_transpose(
                        kvT[:, :, cs * P:(cs + 1) * P], kv_bf[:, cs, :]
                    )

                acc_q = ps_b.tile([P, DT_W], F32, tag="pb")
                acc_k = ps_b.tile([P, DT_W], F32, tag="pb")
                acc_v = ps_b.tile([P, DT_W], F32, tag="pb")
                for mc in range(N_MC):
                    nc.tensor.matmul(
                        acc_q[:, 0:CT_W], lhsT=wq_sb[:, mc], rhs=qT[:, mc, :],
                        start=(mc == 0), stop=(mc == N_MC - 1),
                    )
                    nc.tensor.matmul(
                        acc_k[:, 0:CT_W], lhsT=wk_sb[:, mc], rhs=kvT[:, mc, :],
                        start=(mc == 0), stop=(mc == N_MC - 1),
                    )
                    nc.tensor.matmul(
                        acc_v[:, 0:CT_W], lhsT=wv_sb[:, mc], rhs=kvT[:, mc, :],
                        start=(mc == 0), stop=(mc == N_MC - 1),
                    )
                nc.vector.tensor_copy(qvalT[:, c0:c0 + CT_W], acc_q[:, 0:CT_W])
                nc.vector.tensor_copy(kvalT[:, c0:c0 + CT_W], acc_k[:, 0:CT_W])
                nc.vector.tensor_copy(vvalT[:, c0:c0 + CT_W], acc_v[:, 0:CT_W])

            # vvalT [v2h, c] -> vval [c, v] per head via XBAR transpose.
            vvs = work2.tile([P, N_CC, P], BF16, tag="vvs")
            nc.scalar.dma_start_transpose(vvs, vvalT)
            nc.vector.tensor_copy(vv[:, :, 0:V], vvs[:, :, 0:V])
            nc.vector.tensor_copy(vv[:, :, V + 1:2 * V + 1], vvs[:, :, V:2 * V])

            # ---------- Phase B: attention + output projection ----------
            for dt in range(N_DT):
                d0 = dt * DT_W
                pre0 = ps_b.tile([P, DT_W], F32, tag="pb")  # [v|sum, d] head0
                pre1 = ps_b.tile([P, DT_W], F32, tag="pb")

                ccs = [cc for cc in range(N_CC)
                       if not (SKIP_MASKED and dt < N_DT - 1
                               and _tile_class(cc, dt)[0] == "masked")]

                def emit_s(cc):
                    s01 = ps_s.tile([P, 1024], F32, tag="s")
                    nc.tensor.matmul(
                        s01[:, 0:DT_W],
                        lhsT=kvalT[0:K, cc * P:(cc + 1) * P],
                        rhs=qvalT[0:K, d0:d0 + DT_W],
                        start=True, stop=True,
                    )
                    nc.tensor.matmul(
                        s01[:, DT_W:2 * DT_W],
                        lhsT=kvalT[K:2 * K, cc * P:(cc + 1) * P],
                        rhs=qvalT[K:2 * K, d0:d0 + DT_W],
                        start=True, stop=True,
                    )
                    return s01

                # software pipeline: S(cc+1) is emitted (and thus issued on
                # PE) before AV(cc), so PE computes the next tile's scores
                # while ACT runs exp on the current one.
                s_next = emit_s(ccs[0])
                for i, cc in enumerate(ccs):
                    cls, mi = _tile_class(cc, dt)
                    s01 = s_next
                    if i + 1 < len(ccs):
                        s_next = emit_s(ccs[i + 1])
                    if cls == "cross":
                        s2 = s01.rearrange("p (g d) -> p g d", g=2)
                        nc.vector.tensor_tensor(
                            s2, s2,
                            masks_sb[:, mi, None, :].to_broadcast((P, 2, DT_W)),
                            ADD,
                        )
                    bias = mbias if cls == "masked" else 0.0
                    p01 = work.tile([P, 1024], BF16, tag="p01")
                    nc.scalar.activation(p01, s01, EXP, bias=bias, scale=SCALE)

                    first, last = (i == 0), (i == len(ccs) - 1)
                    nc.tensor.matmul(
                        pre0[0:V + 1, :],
                        lhsT=vv[:, cc, 0:V + 1],
                        rhs=p01[:, 0:DT_W],
                        start=first, stop=last,
                    )
                    nc.tensor.matmul(
                        pre1[0:V + 1, :],
                        lhsT=vv[:, cc, V + 1:2 * V + 2],
                        rhs=p01[:, DT_W:2 * DT_W],
                        start=first, stop=last,
                    )

                # normalize: preT[v, d] = pre[v, d] / pre[V, d]
                srow = work2.tile([P, 2, DT_W], F32R, tag="srow")
                nc.vector.tensor_copy(srow[V:V + 1, 0, :], pre0[V:V + 1, :])
                nc.vector.tensor_copy(srow[V:V + 1, 1, :], pre1[V:V + 1, :])
                for h, (pre_ps, dst_lo) in enumerate(((pre0, True), (pre1, False))):
                    bc = ps_b.tile([P, DT_W], F32, tag="pb")
                    nc.tensor.matmul(
                        bc[0:V, :],
                        lhsT=ones_row[V:V + 1, :],
                        rhs=srow[V:V + 1, h, :],
                        start=True, stop=True,
                    )
                    rb = work2.tile([P, DT_W], F32, tag="rb")
                    with nc.allow_low_precision(reason="softmax denominator"):
                        nc.vector.reciprocal_approx_fast(rb[0:V, :], bc[0:V, :])
                    if dst_lo:
                        nc.vector.tensor_tensor(
                            preT[0:V, d0:d0 + DT_W], pre_ps[0:V, :], rb[0:V, :], MULT
                        )
                    else:
                        pre1n = work2.tile([P, DT_W], BF16, tag="pre1n")
                        nc.vector.tensor_tensor(
                            pre1n[0:V, :], pre_ps[0:V, :], rb[0:V, :], MULT
                        )
                        # head1 rows live at partitions 0..63; shift to 64..127
                        nc.sync.dma_start(preT[V:2 * V, d0:d0 + DT_W], pre1n[0:V, :])

            # output projection: out[d, m] = sum_{v2h} preT[v2h, d] * wo[v2h, m]
            # Split into d-halves; each half's ReduceScatter is kicked as soon
            # as its output rows are in DRAM, overlapping the collective with
            # the remaining compute.
            pb2 = partial[b].rearrange("(half dd p mm) -> half p dd mm", half=2, p=P, mm=M)
            half_sub = C // P // 2
            for half in range(2):
                for dsl in range(half_sub):
                    ds_ = half * half_sub + dsl
                    for mt in range(M // DT_W):
                        o_ps = ps_b.tile([P, DT_W], F32, tag="pb")
                        nc.tensor.matmul(
                            o_ps,
                            lhsT=preT[:, ds_ * P:(ds_ + 1) * P],
                            rhs=wo_sb[:, mt * DT_W:(mt + 1) * DT_W],
                            start=True, stop=True,
                        )
                        o_sb = work.tile([P, DT_W], F32, tag="o_sb")
                        if mt % 2 == 0:
                            nc.vector.tensor_copy(o_sb, o_ps)
                        else:
                            nc.scalar.copy(o_sb, o_ps)
                        nc.sync.dma_start(
                            pb2[half, :, dsl, mt * DT_W:(mt + 1) * DT_W], o_sb
                        )
                hsz = C * M // 2
                ssz = hsz // NCORES
                nc.gpsimd.collective_compute(
                    "ReduceScatter",
                    ADD,
                    replica_groups=[list(range(NCORES))],
                    ins=[partial[b, half * hsz:(half + 1) * hsz].opt()],
                    outs=[shard[b, half * ssz:(half + 1) * ssz].opt()],
                )
                nc.gpsimd.dma_start(
                    out_d[b, half * ssz:(half + 1) * ssz],
                    shard[b, half * ssz:(half + 1) * ssz],
                )

    nc.finalize()
    return nc


_CACHE: dict = {}


def run(inputs: dict, trace: bool = False, **spmd_kwargs):
    """Run on 8 cores; returns (BassKernelResults, assembled output)."""
    if "nc" not in _CACHE:
        _CACHE["nc"] = build_module()
    nc = _CACHE["nc"]

    kvinput = np.ascontiguousarray(np.asarray(inputs["kvinput"], dtype=np.float32))
    qinput = np.ascontiguousarray(np.asarray(inputs["qinput"], dtype=np.float32))
    wq = np.asarray(inputs["wq"], dtype=np.float32)
    wk = np.asarray(inputs["wk"], dtype=np.float32)
    wv = np.asarray(inputs["wv"], dtype=np.float32)
    wo = np.asarray(inputs["wo"], dtype=np.float32)
    masks = make_masks()

    in_maps = []
    for i in range(NCORES):
        h0 = i * HPC
        in_maps.append({
            "kvinput": kvinput,
            "qinput": qinput,
            "wq": np.ascontiguousarray(wq[h0:h0 + HPC]),
            "wk": np.ascontiguousarray(wk[h0:h0 + HPC]),
            "wv": np.ascontiguousarray(wv[h0:h0 + HPC]),
            "wo": np.ascontiguousarray(wo[h0:h0 + HPC]),
            "masks": masks,
        })

    res = run_bass_kernel_spmd(
        nc, in_maps, core_ids=list(range(NCORES)), trace=trace, **spmd_kwargs
    )
    # Each rank's "out" is [B, CM/8] = [B, 2 halves, CM/16]: slice r of each
    # half-batch ReduceScatter.
    qsz = C * M // (2 * NCORES)
    out = np.empty((B, 2, NCORES, qsz), dtype=np.float32)
    for r in range(NCORES):
        out[:, :, r, :] = res.results[r]["out"].reshape(B, 2, qsz)
    return res, out.reshape(B, C, M)


def kernel(**inputs: np.ndarray) -> np.ndarray:
    return run(inputs)[1]
